# revision 40
# baseline (speedup 1.0000x reference)
"""Multi-head attention (GAttention) on 8 trn2 NeuronCores — v3.

Reference computation (per batch b):
    q = x @ w_qkv.T            -> [N, 768], heads of 64
    attn = softmax(q k^T / 8)  -> per head [N, M]
    out_h = attn @ v           -> [N, 64]
    out = concat(out_h) @ w_proj.T + b_proj

Sharding: 24 (b, head) units over 8 cores -> each core one batch b and 3
heads; cores emit f16 partial projections [N, 768]; host sums 4 partials
per batch + bias.

v3 design (PE-bound; every engine rebalanced around the attention
matmul stream):
  * exp splits between the Scalar (ACT) engine and a CUSTOM DVE op
    "EXP32Q_ANT": e^x ~ ((t+A)t+B)^32, one 8-stage Vector-engine
    instruction (quadratic Horner + 5 squarings).  sigma = 0.125/(32 kk)
    is folded into w_q host-side; the ACT path uses activation scale
    32 kk.
  * heads 0/1 are PAIRED on the PE: qproj computes both in one output
    tile (rows 0:64 = h0, 64:128 = h1, no duplication), and S^T packs
    (h0, h1) of the same m-tile on PE row groups 0/64.  Head 2 uses the
    m-pair packing with duplicated q rows.  This cuts qproj matmuls by
    a third.
  * one flat phase: qproj(h01, ch0) runs first; all other qproj chunks
    and each completed n-quarter's projection are injected into the
    attention stream in <=3-matmul pieces so the S^T->EXP cadence never
    stalls.
  * PSUM (8 banks): st 2x[128,2,512] (4) + av 2x[128,512] (2: the h01
    unit's two accumulators) + one shared [128,2,512] ring (2) used by
    qproj chunks, proj groups, and the h2 unit's accumulator.
  * proj packs n-tile pairs into one [128,2,384] PSUM tile; one engine
    copy casts to f16, two DMAs store.
  * outTn row duplication is a SBUF->SBUF DMA, not a second multiply.
  * inputs host-packed partition-major, DMA order == need order, issue
    spread over the SP and ACT queues.
"""
import numpy as np
import ml_dtypes
from contextlib import ExitStack

import concourse.bass as bass
import concourse.mybir as mybir
import concourse.tile as tile
from concourse import bacc
from concourse.bass_utils import run_bass_kernel_spmd

B, N, DIM = 2, 2048, 768
H, D = 12, 64
M = 2048
NCORES = 8
HPC = 3
NT = N // 128
MT = M // 128
MP = MT // 2
CT = DIM // 128
NQ = 4
QW = N // NQ
F32 = mybir.dt.float32
F16 = mybir.dt.float16
BF16 = mybir.dt.bfloat16

KK = 1.33947417
EA = 1.33152807
EB = 1.00021259
ACT_SCALE = 32.0 * KK
SIGMA = 0.125 / (32.0 * KK)

import os
DVE_PER16 = int(os.environ.get("KX", "5"))
_DVE_MOD = set(round(i * 16 / max(DVE_PER16, 1) + 1) % 16
               for i in range(DVE_PER16))
LAG = 2

_cached = {}


def _register_exp_op():
    import concourse.dve_ops as dvo
    from concourse.dve_spec import Spec, Src0, C0, C1, sq, lower
    from concourse.dve_uop import DveOpSpec

    name = "EXP32Q_ANT"
    for op in dvo.OPS:
        if op.name == name:
            return op
    p = (Src0 + C0) * Src0 + C1
    for _ in range(5):
        p = sq(p)
    spec = Spec(body=p)
    row = max(dvo._SUB_OPCODE_FOR_NAME.values()) + 1
    assert row < 0x20
    dvo._SUB_OPCODE_FOR_NAME[name] = row
    shas = {}
    for ver in ("v3", "v4"):
        try:
            uops = lower(spec, ver=ver)
            shas[ver] = DveOpSpec(
                name=name, opcode=row, uops=uops, rd1_en=False).sha(ver)
        except Exception:
            pass
    op = dvo.DveOp(name, spec, subdim=False, uops_sha=shas)
    dvo.OPS.append(op)
    dvo.CUSTOM_DVE_SPECS[name] = spec
    return op


def build_program():
    exp_op = _register_exp_op()
    nc = bacc.Bacc("TRN2", target_bir_lowering=False, debug=False)
    xT_d = nc.dram_tensor("xT", [DIM, N], BF16, kind="ExternalInput")
    # wq slot 0 = (h0 cols | h1 cols), slot 1 = h2 duplicated
    wq_d = nc.dram_tensor("wq", [128, 2, CT, 128], BF16,
                          kind="ExternalInput")
    # kT rows 0:16 = h01-paired m-tiles, rows 16:24 = h2 m-pairs
    kT_d = nc.dram_tensor("kT", [128, MT + MP, 128], BF16,
                          kind="ExternalInput")
    va_d = nc.dram_tensor("va", [128, HPC, MT, 128], BF16,
                          kind="ExternalInput")
    wp_d = nc.dram_tensor("wp", [128, HPC, DIM], BF16, kind="ExternalInput")
    out_d = nc.dram_tensor("out", [N, DIM], F16, kind="ExternalOutput")

    with tile.TileContext(nc) as tc, ExitStack() as ctx:
        big = ctx.enter_context(tc.tile_pool(name="big", bufs=1))
        etp = ctx.enter_context(tc.tile_pool(name="etp", bufs=6))
        rsp = ctx.enter_context(tc.tile_pool(name="rsp", bufs=2))
        stg = ctx.enter_context(tc.tile_pool(name="stg", bufs=3))

        wq_t = big.tile([128, 2, CT, 128], BF16)
        nc.scalar.dma_start(wq_t[:, 0:1, :, :], wq_d[:, 0:1, :, :])
        xT_t = [big.tile([128, N], BF16, name=f"xT{c}", tag=f"xT{c}")
                for c in range(CT)]
        for c in range(CT):
            eng = nc.sync if c % 2 == 0 else nc.scalar
            eng.dma_start(xT_t[c][:, 0:QW],
                          xT_d[c * 128:(c + 1) * 128, 0:QW])
        kT_t = big.tile([128, MT + MP, 128], BF16)
        nc.sync.dma_start(kT_t[:, 0:4, :], kT_d[:, 0:4, :])
        va_t = big.tile([128, HPC, MT, 128], BF16)
        nc.sync.dma_start(va_t[:, :, 0:4, :], va_d[:, :, 0:4, :])
        nc.scalar.dma_start(wq_t[:, 1:2, :, :], wq_d[:, 1:2, :, :])
        nc.sync.dma_start(kT_t[:, 4:MT + MP, :], kT_d[:, 4:MT + MP, :])
        nc.sync.dma_start(va_t[:, :, 4:MT, :], va_d[:, :, 4:MT, :])
        for c in range(CT):
            nc.sync.dma_start(xT_t[c][:, QW:N],
                              xT_d[c * 128:(c + 1) * 128, QW:N])
        wp_t = big.tile([128, HPC, DIM], BF16)
        nc.sync.dma_start(wp_t[:], wp_d[:])

        qT_t = big.tile([128, 2, N], BF16)
        outTn_t = big.tile([128, HPC, N], BF16)
        zz_t = big.tile([128, 64], BF16)
        nc.vector.memset(zz_t[:], 0.0)

        acc_ps = ctx.enter_context(
            tc.tile_pool(name="acc_ps", bufs=2, space="PSUM"))
        st_stack = ExitStack()
        st_ps = st_stack.enter_context(
            tc.tile_pool(name="st_ps", bufs=2, space="PSUM"))

        qp_live = {}

        def qp_piece(s, ch, second):
            nsl = slice(ch * QW, (ch + 1) * QW)
            if not second:
                qp_live[(s, ch)] = acc_ps.tile(
                    [128, 2, QW], F32, tag="acc", name="acc", bufs=1)
            qp = qp_live[(s, ch)]
            for c in (range(3, CT) if second else range(3)):
                nc.tensor.matmul(
                    qp[:, 0, :], wq_t[:, s, c, :], xT_t[c][:, nsl],
                    start=(c == 0), stop=(c == CT - 1),
                )
            if second:
                nc.vector.tensor_copy(qT_t[:, s, nsl], qp[:, 0, :])
                del qp_live[(s, ch)]

        pj_live = {}

        def pj_piece(q, j, oc, second, pool, nbufs=1, dma_split=False):
            na = (q * 4 + 2 * j) * 128
            nb = na + 128
            osl = slice(oc * 384, (oc + 1) * 384)
            if not second:
                pj_live[(q, j, oc)] = pool.tile(
                    [128, 2, QW], F32, tag="acc", name="acc", bufs=nbufs)
            pjv = pj_live[(q, j, oc)][:, :, 0:384]
            for hh in ((2,) if second else (0, 1)):
                nc.tensor.matmul(
                    pjv[:, 0, :], outTn_t[0:64, hh, na:na + 128],
                    wp_t[0:64, hh, osl],
                    start=(hh == 0), stop=(hh == HPC - 1),
                    tile_position=(0, 0),
                )
                nc.tensor.matmul(
                    pjv[:, 1, :], outTn_t[64:128, hh, nb:nb + 128],
                    wp_t[64:128, hh, osl],
                    start=(hh == 0), stop=(hh == HPC - 1),
                    tile_position=(64, 0),
                )
            if second:
                ot = stg.tile([128, 2, 384], F16, tag="ot", name="ot")
                if (j + oc) % 2 == 0:
                    nc.scalar.copy(ot[:], pjv[:])
                else:
                    nc.vector.tensor_copy(ot[:], pjv[:])
                eng2 = nc.scalar if dma_split else nc.sync
                nc.sync.dma_start(out_d[na:na + 128, osl], ot[:, 0, :])
                eng2.dma_start(out_d[nb:nb + 128, osl], ot[:, 1, :])
                del pj_live[(q, j, oc)]

        # schedule: per 24-iter q-block, iters 0:16 are the h01-paired
        # unit and 16:24 the h2 unit.  The shared "acc" ring holds the h2
        # accumulator from block-iter 16 until the next block's iter ~1,
        # so injected pieces (qproj / proj) sit only in block slots 2..15.
        inject = {}

        def put(g, piece):
            assert g not in inject and 2 <= (g % 24) <= 15, g
            inject[g] = piece

        put(2, ("qp", 1, 0, False))
        put(3, ("qp", 1, 0, True))
        put(12, ("qp", 0, 1, False))
        put(13, ("qp", 0, 1, True))
        put(14, ("qp", 1, 1, False))
        put(15, ("qp", 1, 1, True))
        for q in range(NQ - 1):
            b = 24 * (q + 1) + 3
            for g in range(4):
                j, oc = g // 2, g % 2
                put(b + 2 * g, ("pj", q, j, oc, False))
                put(b + 2 * g + 1, ("pj", q, j, oc, True))
        put(35, ("qp", 0, 2, False))
        put(36, ("qp", 0, 2, True))
        put(37, ("qp", 1, 2, False))
        put(38, ("qp", 1, 2, True))
        put(59, ("qp", 0, 3, False))
        put(60, ("qp", 0, 3, True))
        put(61, ("qp", 1, 3, False))
        put(62, ("qp", 1, 3, True))

        av_by_key = {}
        pend = []

        def _av(pd):
            kind, q, idx, et, first, last = pd
            if kind == "P":
                nc.tensor.matmul(av_by_key[("P0", q)][:],
                                 va_t[:, 0, idx, :], et[:, 0, :],
                                 start=first, stop=last)
                nc.tensor.matmul(av_by_key[("P1", q)][:],
                                 va_t[:, 1, idx, :], et[:, 1, :],
                                 start=first, stop=last)
            else:
                av2 = av_by_key[("S", q)][:, 0, :]
                nc.tensor.matmul(av2, va_t[:, 2, 2 * idx, :], et[:, 0, :],
                                 start=first, stop=False)
                nc.tensor.matmul(av2, va_t[:, 2, 2 * idx + 1, :],
                                 et[:, 1, :], start=False, stop=last)

        def _norm_one(av, h, q):
            nsl = slice(q * QW, (q + 1) * QW)
            rs = rsp.tile([64, QW], F32, tag="rs", name="rs")
            nc.vector.reciprocal_approx_fast(rs[:], av[0:64, :])
            nc.vector.tensor_mul(outTn_t[0:64, h, nsl], av[64:128, :],
                                 rs[:])
            nc.sync.dma_start(outTn_t[64:128, h, nsl],
                              outTn_t[0:64, h, nsl])

        def _norm(kind, q):
            if kind == "P":
                _norm_one(av_by_key[("P0", q)][:], 0, q)
                _norm_one(av_by_key[("P1", q)][:], 1, q)
            else:
                _norm_one(av_by_key[("S", q)][:, 0, :], 2, q)

        def _flush(limit):
            while len(pend) > limit:
                pd = pend.pop(0)
                _av(pd)
                if pd[5]:
                    _norm(pd[0], pd[1])

        for w in range(2):
            wt = st_ps.tile([128, 2, QW], F32, tag="st", name="st")
            for i in range(20):
                nc.tensor.matmul(wt[0:64, 0, 0:64], zz_t[:, 0:64],
                                 zz_t[:, 0:64], start=True, stop=True)

        qp_piece(0, 0, False)
        qp_piece(0, 0, True)

        gi = 0
        for q in range(NQ):
            nsl = slice(q * QW, (q + 1) * QW)
            for kind, niter in (("P", MT), ("S", MP)):
                for it in range(niter):
                    if it == 0:
                        if kind == "P":
                            av_by_key[("P0", q)] = acc_ps.tile(
                                [128, QW], F32, tag="av", name="av")
                            av_by_key[("P1", q)] = acc_ps.tile(
                                [128, QW], F32, tag="av", name="av")
                        else:
                            av_by_key[("S", q)] = acc_ps.tile(
                                [128, 2, QW], F32, tag="acc", name="acc",
                                bufs=1)
                    kt = it if kind == "P" else MT + it
                    qs = 0 if kind == "P" else 1
                    st = st_ps.tile([128, 2, QW], F32, tag="st", name="st")
                    nc.tensor.matmul(
                        st[:, 0, :], kT_t[0:64, kt, :],
                        qT_t[0:64, qs, nsl],
                        start=True, stop=True, tile_position=(0, 0),
                    )
                    nc.tensor.matmul(
                        st[:, 1, :], kT_t[64:128, kt, :],
                        qT_t[64:128, qs, nsl],
                        start=True, stop=True, tile_position=(64, 0),
                    )
                    _flush(LAG - 1)
                    et = etp.tile([128, 2, QW], BF16, tag="et", name="et")
                    if (gi % 16) in _DVE_MOD and gi < 88:
                        nc.vector._custom_dve(
                            exp_op, out=et[:], in0=st[:],
                            s0=float(EA), s1=float(EB))
                    else:
                        nc.scalar.activation(
                            et[:], st[:], mybir.ActivationFunctionType.Exp,
                            scale=ACT_SCALE)
                    pend.append((kind, q, it, et, it == 0,
                                 it == niter - 1))
                    item = inject.get(gi)
                    if item is not None:
                        if item[0] == "qp":
                            qp_piece(item[1], item[2], item[3])
                        else:
                            pj_piece(item[1], item[2], item[3], item[4],
                                     acc_ps)
                    gi += 1
        _flush(0)
        st_stack.close()
        with tc.tile_pool(name="tail_ps", bufs=2, space="PSUM") as tailp:
            for jj in range(2):
                pj_piece(NQ - 1, jj, 0, False, tailp, 2)
                pj_piece(NQ - 1, jj, 1, False, tailp, 2)
                pj_piece(NQ - 1, jj, 0, True, tailp, 2, dma_split=True)
                pj_piece(NQ - 1, jj, 1, True, tailp, 2, dma_split=True)

    nc.compile()
    return nc


def build_in_maps(x, k, v, w_qkv, w_proj):
    x = np.asarray(x, dtype=np.float32)
    k = np.asarray(k, dtype=np.float32)
    v = np.asarray(v, dtype=np.float32)
    wqT = np.ascontiguousarray(np.asarray(w_qkv, np.float32).T) * SIGMA
    wpT = np.ascontiguousarray(np.asarray(w_proj, np.float32).T)
    bf = ml_dtypes.bfloat16

    in_maps = []
    for core in range(NCORES):
        b = core // 4
        hs = [3 * (core % 4) + i for i in range(HPC)]
        xT = np.ascontiguousarray(x[b].T.astype(bf))
        # wq slot 0: h0 cols 0:64, h1 cols 64:128; slot 1: h2 duplicated
        wq = np.empty((128, 2, CT, 128), dtype=bf)
        for sl, (ha, hb) in enumerate(((hs[0], hs[1]), (hs[2], hs[2]))):
            ba = wqT[:, 64 * ha:64 * (ha + 1)].reshape(CT, 128, 64)
            bb = wqT[:, 64 * hb:64 * (hb + 1)].reshape(CT, 128, 64)
            wq[:, sl, :, 0:64] = ba.transpose(1, 0, 2).astype(bf)
            wq[:, sl, :, 64:128] = bb.transpose(1, 0, 2).astype(bf)
        # kT: entries 0:16 pair h0 (rows 0:64) with h1 (rows 64:128) per
        # m-tile; entries 16:24 pack h2's even/odd m-tiles
        kT = np.empty((128, MT + MP, 128), dtype=bf)
        k0, k1, k2 = (k[b, hs[0]], k[b, hs[1]], k[b, hs[2]])
        for t in range(MT):
            kT[0:64, t, :] = k0[128 * t:128 * (t + 1), :].T
            kT[64:128, t, :] = k1[128 * t:128 * (t + 1), :].T
        for p in range(MP):
            kT[0:64, MT + p, :] = k2[256 * p:256 * p + 128, :].T
            kT[64:128, MT + p, :] = k2[256 * p + 128:256 * p + 256, :].T
        # va: cols 0:64 ones (denominator), 64:128 = v
        va = np.ones((128, HPC, MT, 128), dtype=bf)
        for hi, h in enumerate(hs):
            va[:, hi, :, D:2 * D] = v[b, h].reshape(MT, 128, D).transpose(
                1, 0, 2).astype(bf)
        wp = np.empty((128, HPC, DIM), dtype=bf)
        for hi, h in enumerate(hs):
            wp[0:64, hi, :] = wpT[64 * h:64 * (h + 1), :].astype(bf)
            wp[64:128, hi, :] = wp[0:64, hi, :]
        in_maps.append({"xT": xT, "wq": np.ascontiguousarray(wq),
                        "kT": np.ascontiguousarray(kT),
                        "va": np.ascontiguousarray(va),
                        "wp": np.ascontiguousarray(wp)})
    return in_maps


def kernel(x, k, v, w_qkv, w_proj, b_proj):
    b_proj = np.asarray(b_proj, dtype=np.float32)

    if "nc" not in _cached:
        _cached["nc"] = build_program()
    nc = _cached["nc"]

    in_maps = build_in_maps(x, k, v, w_qkv, w_proj)
    res = run_bass_kernel_spmd(nc, in_maps, core_ids=list(range(NCORES)))

    out = np.empty((B, N, DIM), dtype=np.float32)
    for b in range(B):
        acc = np.zeros((N, DIM), dtype=np.float64)
        for core in range(4 * b, 4 * b + 4):
            acc += res.results[core]["out"].astype(np.float64)
        out[b] = (acc + b_proj).astype(np.float32)
    return out


# revision 41
# speedup vs baseline: 1.0253x; 1.0253x over previous
"""Multi-head attention (GAttention) on 8 trn2 NeuronCores — v3.

Reference computation (per batch b):
    q = x @ w_qkv.T            -> [N, 768], heads of 64
    attn = softmax(q k^T / 8)  -> per head [N, M]
    out_h = attn @ v           -> [N, 64]
    out = concat(out_h) @ w_proj.T + b_proj

Sharding: 24 (b, head) units over 8 cores -> each core one batch b and 3
heads; cores emit f16 partial projections [N, 768]; host sums 4 partials
per batch + bias.

v3 design (PE-bound; every engine rebalanced around the attention
matmul stream):
  * exp splits between the Scalar (ACT) engine and a CUSTOM DVE op
    "EXP32Q_ANT": e^x ~ ((t+A)t+B)^32, one 8-stage Vector-engine
    instruction (quadratic Horner + 5 squarings).  sigma = 0.125/(32 kk)
    is folded into w_q host-side; the ACT path uses activation scale
    32 kk.
  * heads 0/1 are PAIRED on the PE: qproj computes both in one output
    tile (rows 0:64 = h0, 64:128 = h1, no duplication), and S^T packs
    (h0, h1) of the same m-tile on PE row groups 0/64.  Head 2 uses the
    m-pair packing with duplicated q rows.  This cuts qproj matmuls by
    a third.
  * one flat phase: qproj(h01, ch0) runs first; all other qproj chunks
    and each completed n-quarter's projection are injected into the
    attention stream in <=3-matmul pieces so the S^T->EXP cadence never
    stalls.
  * PSUM (8 banks): st 2x[128,2,512] (4) + av 2x[128,512] (2: the h01
    unit's two accumulators) + one shared [128,2,512] ring (2) used by
    qproj chunks, proj groups, and the h2 unit's accumulator.
  * proj packs n-tile pairs into one [128,2,384] PSUM tile; one engine
    copy casts to f16, two DMAs store.
  * outTn row duplication is a SBUF->SBUF DMA, not a second multiply.
  * inputs host-packed partition-major, DMA order == need order, issue
    spread over the SP and ACT queues.
"""
import numpy as np
import ml_dtypes
from contextlib import ExitStack

import concourse.bass as bass
import concourse.mybir as mybir
import concourse.tile as tile
from concourse import bacc
from concourse.bass_utils import run_bass_kernel_spmd

B, N, DIM = 2, 2048, 768
H, D = 12, 64
M = 2048
NCORES = 8
HPC = 3
NT = N // 128
MT = M // 128
MP = MT // 2
CT = DIM // 128
NQ = 4
QW = N // NQ
F32 = mybir.dt.float32
F16 = mybir.dt.float16
BF16 = mybir.dt.bfloat16

KK = 1.33947417
EA = 1.33152807
EB = 1.00021259
ACT_SCALE = 32.0 * KK
SIGMA = 0.125 / (32.0 * KK)

import os
DVE_PER16 = int(os.environ.get("KX", "5"))
_DVE_MOD = set(round(i * 16 / max(DVE_PER16, 1) + 1) % 16
               for i in range(DVE_PER16))
LAG = 3

_cached = {}


def _register_exp_op():
    import concourse.dve_ops as dvo
    from concourse.dve_spec import Spec, Src0, C0, C1, sq, lower
    from concourse.dve_uop import DveOpSpec

    name = "EXP32Q_ANT"
    for op in dvo.OPS:
        if op.name == name:
            return op
    p = (Src0 + C0) * Src0 + C1
    for _ in range(5):
        p = sq(p)
    spec = Spec(body=p)
    row = max(dvo._SUB_OPCODE_FOR_NAME.values()) + 1
    assert row < 0x20
    dvo._SUB_OPCODE_FOR_NAME[name] = row
    shas = {}
    for ver in ("v3", "v4"):
        try:
            uops = lower(spec, ver=ver)
            shas[ver] = DveOpSpec(
                name=name, opcode=row, uops=uops, rd1_en=False).sha(ver)
        except Exception:
            pass
    op = dvo.DveOp(name, spec, subdim=False, uops_sha=shas)
    dvo.OPS.append(op)
    dvo.CUSTOM_DVE_SPECS[name] = spec
    return op


def build_program():
    exp_op = _register_exp_op()
    nc = bacc.Bacc("TRN2", target_bir_lowering=False, debug=False)
    xT_d = nc.dram_tensor("xT", [DIM, N], BF16, kind="ExternalInput")
    # wq slot 0 = (h0 cols | h1 cols), slot 1 = h2 duplicated
    wq_d = nc.dram_tensor("wq", [128, 2, CT, 128], BF16,
                          kind="ExternalInput")
    # kT rows 0:16 = h01-paired m-tiles, rows 16:24 = h2 m-pairs
    kT_d = nc.dram_tensor("kT", [128, MT + MP, 128], BF16,
                          kind="ExternalInput")
    va_d = nc.dram_tensor("va", [128, HPC, MT, 128], BF16,
                          kind="ExternalInput")
    wp_d = nc.dram_tensor("wp", [128, HPC, DIM], BF16, kind="ExternalInput")
    out_d = nc.dram_tensor("out", [N, DIM], F16, kind="ExternalOutput")

    with tile.TileContext(nc) as tc, ExitStack() as ctx:
        big = ctx.enter_context(tc.tile_pool(name="big", bufs=1))
        etp = ctx.enter_context(tc.tile_pool(name="etp", bufs=6))
        rsp = ctx.enter_context(tc.tile_pool(name="rsp", bufs=2))
        stg = ctx.enter_context(tc.tile_pool(name="stg", bufs=3))

        wq_t = big.tile([128, 2, CT, 128], BF16)
        nc.scalar.dma_start(wq_t[:, 0:1, :, :], wq_d[:, 0:1, :, :])
        xT_t = [big.tile([128, N], BF16, name=f"xT{c}", tag=f"xT{c}")
                for c in range(CT)]
        for c in range(CT):
            eng = nc.sync if c % 2 == 0 else nc.scalar
            eng.dma_start(xT_t[c][:, 0:QW],
                          xT_d[c * 128:(c + 1) * 128, 0:QW])
        kT_t = big.tile([128, MT + MP, 128], BF16)
        nc.sync.dma_start(kT_t[:, 0:4, :], kT_d[:, 0:4, :])
        va_t = big.tile([128, HPC, MT, 128], BF16)
        nc.sync.dma_start(va_t[:, :, 0:4, :], va_d[:, :, 0:4, :])
        nc.scalar.dma_start(wq_t[:, 1:2, :, :], wq_d[:, 1:2, :, :])
        nc.sync.dma_start(kT_t[:, 4:MT + MP, :], kT_d[:, 4:MT + MP, :])
        nc.sync.dma_start(va_t[:, :, 4:MT, :], va_d[:, :, 4:MT, :])
        for c in range(CT):
            nc.sync.dma_start(xT_t[c][:, QW:N],
                              xT_d[c * 128:(c + 1) * 128, QW:N])
        wp_t = big.tile([128, HPC, DIM], BF16)
        nc.sync.dma_start(wp_t[:], wp_d[:])

        qT_t = big.tile([128, 2, N], BF16)
        outTn_t = big.tile([128, HPC, N], BF16)
        zz_t = big.tile([128, 64], BF16)
        nc.vector.memset(zz_t[:], 0.0)

        acc_ps = ctx.enter_context(
            tc.tile_pool(name="acc_ps", bufs=2, space="PSUM"))
        st_stack = ExitStack()
        st_ps = st_stack.enter_context(
            tc.tile_pool(name="st_ps", bufs=2, space="PSUM"))

        qp_live = {}

        def qp_piece(s, ch, second):
            nsl = slice(ch * QW, (ch + 1) * QW)
            if not second:
                qp_live[(s, ch)] = acc_ps.tile(
                    [128, 2, QW], F32, tag="acc", name="acc", bufs=1)
            qp = qp_live[(s, ch)]
            for c in (range(3, CT) if second else range(3)):
                nc.tensor.matmul(
                    qp[:, 0, :], wq_t[:, s, c, :], xT_t[c][:, nsl],
                    start=(c == 0), stop=(c == CT - 1),
                )
            if second:
                nc.vector.tensor_copy(qT_t[:, s, nsl], qp[:, 0, :])
                del qp_live[(s, ch)]

        pj_live = {}

        def pj_piece(q, j, oc, second, pool, nbufs=1, dma_split=False):
            na = (q * 4 + 2 * j) * 128
            nb = na + 128
            osl = slice(oc * 384, (oc + 1) * 384)
            if not second:
                pj_live[(q, j, oc)] = pool.tile(
                    [128, 2, QW], F32, tag="acc", name="acc", bufs=nbufs)
            pjv = pj_live[(q, j, oc)][:, :, 0:384]
            for hh in ((2,) if second else (0, 1)):
                nc.tensor.matmul(
                    pjv[:, 0, :], outTn_t[0:64, hh, na:na + 128],
                    wp_t[0:64, hh, osl],
                    start=(hh == 0), stop=(hh == HPC - 1),
                    tile_position=(0, 0),
                )
                nc.tensor.matmul(
                    pjv[:, 1, :], outTn_t[64:128, hh, nb:nb + 128],
                    wp_t[64:128, hh, osl],
                    start=(hh == 0), stop=(hh == HPC - 1),
                    tile_position=(64, 0),
                )
            if second:
                ot = stg.tile([128, 2, 384], F16, tag="ot", name="ot")
                if (j + oc) % 2 == 0:
                    nc.scalar.copy(ot[:], pjv[:])
                else:
                    nc.vector.tensor_copy(ot[:], pjv[:])
                eng2 = nc.scalar if dma_split else nc.sync
                nc.sync.dma_start(out_d[na:na + 128, osl], ot[:, 0, :])
                eng2.dma_start(out_d[nb:nb + 128, osl], ot[:, 1, :])
                del pj_live[(q, j, oc)]

        # schedule: per 24-iter q-block, iters 0:16 are the h01-paired
        # unit and 16:24 the h2 unit.  The shared "acc" ring holds the h2
        # accumulator from block-iter 16 until the next block's iter ~1,
        # so injected pieces (qproj / proj) sit only in block slots 2..15.
        inject = {}

        def put(g, piece):
            assert g not in inject and 2 <= (g % 24) <= 15, g
            inject[g] = piece

        put(2, ("qp", 1, 0, False))
        put(3, ("qp", 1, 0, True))
        put(12, ("qp", 0, 1, False))
        put(13, ("qp", 0, 1, True))
        put(14, ("qp", 1, 1, False))
        put(15, ("qp", 1, 1, True))
        for q in range(NQ - 1):
            b = 24 * (q + 1) + 3
            for g in range(4):
                j, oc = g // 2, g % 2
                put(b + 2 * g, ("pj", q, j, oc, False))
                put(b + 2 * g + 1, ("pj", q, j, oc, True))
        put(35, ("qp", 0, 2, False))
        put(36, ("qp", 0, 2, True))
        put(37, ("qp", 1, 2, False))
        put(38, ("qp", 1, 2, True))
        put(59, ("qp", 0, 3, False))
        put(60, ("qp", 0, 3, True))
        put(61, ("qp", 1, 3, False))
        put(62, ("qp", 1, 3, True))

        av_by_key = {}
        pend = []

        def _av(pd):
            kind, q, idx, et, first, last = pd
            if kind == "P":
                nc.tensor.matmul(av_by_key[("P0", q)][:],
                                 va_t[:, 0, idx, :], et[:, 0, :],
                                 start=first, stop=last)
                nc.tensor.matmul(av_by_key[("P1", q)][:],
                                 va_t[:, 1, idx, :], et[:, 1, :],
                                 start=first, stop=last)
            else:
                av2 = av_by_key[("S", q)][:, 0, :]
                nc.tensor.matmul(av2, va_t[:, 2, 2 * idx, :], et[:, 0, :],
                                 start=first, stop=False)
                nc.tensor.matmul(av2, va_t[:, 2, 2 * idx + 1, :],
                                 et[:, 1, :], start=False, stop=last)

        def _norm_one(av, h, q):
            nsl = slice(q * QW, (q + 1) * QW)
            rs = rsp.tile([64, QW], F32, tag="rs", name="rs")
            nc.vector.reciprocal_approx_fast(rs[:], av[0:64, :])
            nc.vector.tensor_mul(outTn_t[0:64, h, nsl], av[64:128, :],
                                 rs[:])
            nc.sync.dma_start(outTn_t[64:128, h, nsl],
                              outTn_t[0:64, h, nsl])

        def _norm(kind, q):
            if kind == "P":
                _norm_one(av_by_key[("P0", q)][:], 0, q)
                _norm_one(av_by_key[("P1", q)][:], 1, q)
            else:
                _norm_one(av_by_key[("S", q)][:, 0, :], 2, q)

        def _flush(limit):
            while len(pend) > limit:
                pd = pend.pop(0)
                _av(pd)
                if pd[5]:
                    _norm(pd[0], pd[1])

        for w in range(2):
            wt = st_ps.tile([128, 2, QW], F32, tag="st", name="st")
            for i in range(20):
                nc.tensor.matmul(wt[0:64, 0, 0:64], zz_t[:, 0:64],
                                 zz_t[:, 0:64], start=True, stop=True)

        qp_piece(0, 0, False)
        qp_piece(0, 0, True)

        gi = 0
        for q in range(NQ):
            nsl = slice(q * QW, (q + 1) * QW)
            for kind, niter in (("P", MT), ("S", MP)):
                for it in range(niter):
                    if it == 0:
                        if kind == "P":
                            av_by_key[("P0", q)] = acc_ps.tile(
                                [128, QW], F32, tag="av", name="av")
                            av_by_key[("P1", q)] = acc_ps.tile(
                                [128, QW], F32, tag="av", name="av")
                        else:
                            av_by_key[("S", q)] = acc_ps.tile(
                                [128, 2, QW], F32, tag="acc", name="acc",
                                bufs=1)
                    kt = it if kind == "P" else MT + it
                    qs = 0 if kind == "P" else 1
                    st = st_ps.tile([128, 2, QW], F32, tag="st", name="st")
                    nc.tensor.matmul(
                        st[:, 0, :], kT_t[0:64, kt, :],
                        qT_t[0:64, qs, nsl],
                        start=True, stop=True, tile_position=(0, 0),
                    )
                    nc.tensor.matmul(
                        st[:, 1, :], kT_t[64:128, kt, :],
                        qT_t[64:128, qs, nsl],
                        start=True, stop=True, tile_position=(64, 0),
                    )
                    _flush(LAG - 1)
                    et = etp.tile([128, 2, QW], BF16, tag="et", name="et")
                    if (gi % 16) in _DVE_MOD and gi < 88:
                        nc.vector._custom_dve(
                            exp_op, out=et[:], in0=st[:],
                            s0=float(EA), s1=float(EB))
                    else:
                        nc.scalar.activation(
                            et[:], st[:], mybir.ActivationFunctionType.Exp,
                            scale=ACT_SCALE)
                    pend.append((kind, q, it, et, it == 0,
                                 it == niter - 1))
                    item = inject.get(gi)
                    if item is not None:
                        if item[0] == "qp":
                            qp_piece(item[1], item[2], item[3])
                        else:
                            pj_piece(item[1], item[2], item[3], item[4],
                                     acc_ps)
                    gi += 1
        _flush(0)
        st_stack.close()
        with tc.tile_pool(name="tail_ps", bufs=2, space="PSUM") as tailp:
            for jj in range(2):
                pj_piece(NQ - 1, jj, 0, False, tailp, 2)
                pj_piece(NQ - 1, jj, 1, False, tailp, 2)
                pj_piece(NQ - 1, jj, 0, True, tailp, 2, dma_split=True)
                pj_piece(NQ - 1, jj, 1, True, tailp, 2, dma_split=True)

    nc.compile()
    return nc


def build_in_maps(x, k, v, w_qkv, w_proj):
    x = np.asarray(x, dtype=np.float32)
    k = np.asarray(k, dtype=np.float32)
    v = np.asarray(v, dtype=np.float32)
    wqT = np.ascontiguousarray(np.asarray(w_qkv, np.float32).T) * SIGMA
    wpT = np.ascontiguousarray(np.asarray(w_proj, np.float32).T)
    bf = ml_dtypes.bfloat16

    in_maps = []
    for core in range(NCORES):
        b = core // 4
        hs = [3 * (core % 4) + i for i in range(HPC)]
        xT = np.ascontiguousarray(x[b].T.astype(bf))
        # wq slot 0: h0 cols 0:64, h1 cols 64:128; slot 1: h2 duplicated
        wq = np.empty((128, 2, CT, 128), dtype=bf)
        for sl, (ha, hb) in enumerate(((hs[0], hs[1]), (hs[2], hs[2]))):
            ba = wqT[:, 64 * ha:64 * (ha + 1)].reshape(CT, 128, 64)
            bb = wqT[:, 64 * hb:64 * (hb + 1)].reshape(CT, 128, 64)
            wq[:, sl, :, 0:64] = ba.transpose(1, 0, 2).astype(bf)
            wq[:, sl, :, 64:128] = bb.transpose(1, 0, 2).astype(bf)
        # kT: entries 0:16 pair h0 (rows 0:64) with h1 (rows 64:128) per
        # m-tile; entries 16:24 pack h2's even/odd m-tiles
        kT = np.empty((128, MT + MP, 128), dtype=bf)
        k0, k1, k2 = (k[b, hs[0]], k[b, hs[1]], k[b, hs[2]])
        for t in range(MT):
            kT[0:64, t, :] = k0[128 * t:128 * (t + 1), :].T
            kT[64:128, t, :] = k1[128 * t:128 * (t + 1), :].T
        for p in range(MP):
            kT[0:64, MT + p, :] = k2[256 * p:256 * p + 128, :].T
            kT[64:128, MT + p, :] = k2[256 * p + 128:256 * p + 256, :].T
        # va: cols 0:64 ones (denominator), 64:128 = v
        va = np.ones((128, HPC, MT, 128), dtype=bf)
        for hi, h in enumerate(hs):
            va[:, hi, :, D:2 * D] = v[b, h].reshape(MT, 128, D).transpose(
                1, 0, 2).astype(bf)
        wp = np.empty((128, HPC, DIM), dtype=bf)
        for hi, h in enumerate(hs):
            wp[0:64, hi, :] = wpT[64 * h:64 * (h + 1), :].astype(bf)
            wp[64:128, hi, :] = wp[0:64, hi, :]
        in_maps.append({"xT": xT, "wq": np.ascontiguousarray(wq),
                        "kT": np.ascontiguousarray(kT),
                        "va": np.ascontiguousarray(va),
                        "wp": np.ascontiguousarray(wp)})
    return in_maps


def kernel(x, k, v, w_qkv, w_proj, b_proj):
    b_proj = np.asarray(b_proj, dtype=np.float32)

    if "nc" not in _cached:
        _cached["nc"] = build_program()
    nc = _cached["nc"]

    in_maps = build_in_maps(x, k, v, w_qkv, w_proj)
    res = run_bass_kernel_spmd(nc, in_maps, core_ids=list(range(NCORES)))

    out = np.empty((B, N, DIM), dtype=np.float32)
    for b in range(B):
        acc = np.zeros((N, DIM), dtype=np.float64)
        for core in range(4 * b, 4 * b + 4):
            acc += res.results[core]["out"].astype(np.float64)
        out[b] = (acc + b_proj).astype(np.float32)
    return out


# revision 42
# speedup vs baseline: 1.0335x; 1.0080x over previous
"""Multi-head attention (GAttention) on 8 trn2 NeuronCores — v3.

Reference computation (per batch b):
    q = x @ w_qkv.T            -> [N, 768], heads of 64
    attn = softmax(q k^T / 8)  -> per head [N, M]
    out_h = attn @ v           -> [N, 64]
    out = concat(out_h) @ w_proj.T + b_proj

Sharding: 24 (b, head) units over 8 cores -> each core one batch b and 3
heads; cores emit f16 partial projections [N, 768]; host sums 4 partials
per batch + bias.

v3 design (PE-bound; every engine rebalanced around the attention
matmul stream):
  * exp splits between the Scalar (ACT) engine and a CUSTOM DVE op
    "EXP32Q_ANT": e^x ~ ((t+A)t+B)^32, one 8-stage Vector-engine
    instruction (quadratic Horner + 5 squarings).  sigma = 0.125/(32 kk)
    is folded into w_q host-side; the ACT path uses activation scale
    32 kk.
  * heads 0/1 are PAIRED on the PE: qproj computes both in one output
    tile (rows 0:64 = h0, 64:128 = h1, no duplication), and S^T packs
    (h0, h1) of the same m-tile on PE row groups 0/64.  Head 2 uses the
    m-pair packing with duplicated q rows.  This cuts qproj matmuls by
    a third.
  * one flat phase: qproj(h01, ch0) runs first; all other qproj chunks
    and each completed n-quarter's projection are injected into the
    attention stream in <=3-matmul pieces so the S^T->EXP cadence never
    stalls.
  * PSUM (8 banks): st 2x[128,2,512] (4) + av 2x[128,512] (2: the h01
    unit's two accumulators) + one shared [128,2,512] ring (2) used by
    qproj chunks, proj groups, and the h2 unit's accumulator.
  * proj packs n-tile pairs into one [128,2,384] PSUM tile; one engine
    copy casts to f16, two DMAs store.
  * outTn row duplication is a SBUF->SBUF DMA, not a second multiply.
  * inputs host-packed partition-major, DMA order == need order, issue
    spread over the SP and ACT queues.
"""
import numpy as np
import ml_dtypes
from contextlib import ExitStack

import concourse.bass as bass
import concourse.mybir as mybir
import concourse.tile as tile
from concourse import bacc
from concourse.bass_utils import run_bass_kernel_spmd

B, N, DIM = 2, 2048, 768
H, D = 12, 64
M = 2048
NCORES = 8
HPC = 3
NT = N // 128
MT = M // 128
MP = MT // 2
CT = DIM // 128
NQ = 4
QW = N // NQ
F32 = mybir.dt.float32
F16 = mybir.dt.float16
BF16 = mybir.dt.bfloat16

KK = 1.33947417
EA = 1.33152807
EB = 1.00021259
ACT_SCALE = 32.0 * KK
SIGMA = 0.125 / (32.0 * KK)

import os
DVE_PER16 = int(os.environ.get("KX", "5"))
_DVE_MOD = set(round(i * 16 / max(DVE_PER16, 1) + 1) % 16
               for i in range(DVE_PER16))
LAG = 4

_cached = {}


def _register_exp_op():
    import concourse.dve_ops as dvo
    from concourse.dve_spec import Spec, Src0, C0, C1, sq, lower
    from concourse.dve_uop import DveOpSpec

    name = "EXP32Q_ANT"
    for op in dvo.OPS:
        if op.name == name:
            return op
    p = (Src0 + C0) * Src0 + C1
    for _ in range(5):
        p = sq(p)
    spec = Spec(body=p)
    row = max(dvo._SUB_OPCODE_FOR_NAME.values()) + 1
    assert row < 0x20
    dvo._SUB_OPCODE_FOR_NAME[name] = row
    shas = {}
    for ver in ("v3", "v4"):
        try:
            uops = lower(spec, ver=ver)
            shas[ver] = DveOpSpec(
                name=name, opcode=row, uops=uops, rd1_en=False).sha(ver)
        except Exception:
            pass
    op = dvo.DveOp(name, spec, subdim=False, uops_sha=shas)
    dvo.OPS.append(op)
    dvo.CUSTOM_DVE_SPECS[name] = spec
    return op


def build_program():
    exp_op = _register_exp_op()
    nc = bacc.Bacc("TRN2", target_bir_lowering=False, debug=False)
    xT_d = nc.dram_tensor("xT", [DIM, N], BF16, kind="ExternalInput")
    # wq slot 0 = (h0 cols | h1 cols), slot 1 = h2 duplicated
    wq_d = nc.dram_tensor("wq", [128, 2, CT, 128], BF16,
                          kind="ExternalInput")
    # kT rows 0:16 = h01-paired m-tiles, rows 16:24 = h2 m-pairs
    kT_d = nc.dram_tensor("kT", [128, MT + MP, 128], BF16,
                          kind="ExternalInput")
    va_d = nc.dram_tensor("va", [128, HPC, MT, 128], BF16,
                          kind="ExternalInput")
    wp_d = nc.dram_tensor("wp", [128, HPC, DIM], BF16, kind="ExternalInput")
    out_d = nc.dram_tensor("out", [N, DIM], F16, kind="ExternalOutput")

    with tile.TileContext(nc) as tc, ExitStack() as ctx:
        big = ctx.enter_context(tc.tile_pool(name="big", bufs=1))
        etp = ctx.enter_context(tc.tile_pool(name="etp", bufs=6))
        rsp = ctx.enter_context(tc.tile_pool(name="rsp", bufs=2))
        stg = ctx.enter_context(tc.tile_pool(name="stg", bufs=3))

        wq_t = big.tile([128, 2, CT, 128], BF16)
        nc.scalar.dma_start(wq_t[:, 0:1, :, :], wq_d[:, 0:1, :, :])
        xT_t = [big.tile([128, N], BF16, name=f"xT{c}", tag=f"xT{c}")
                for c in range(CT)]
        for c in range(CT):
            eng = nc.sync if c % 2 == 0 else nc.scalar
            eng.dma_start(xT_t[c][:, 0:QW],
                          xT_d[c * 128:(c + 1) * 128, 0:QW])
        kT_t = big.tile([128, MT + MP, 128], BF16)
        nc.sync.dma_start(kT_t[:, 0:4, :], kT_d[:, 0:4, :])
        va_t = big.tile([128, HPC, MT, 128], BF16)
        nc.sync.dma_start(va_t[:, :, 0:4, :], va_d[:, :, 0:4, :])
        nc.scalar.dma_start(wq_t[:, 1:2, :, :], wq_d[:, 1:2, :, :])
        nc.sync.dma_start(kT_t[:, 4:MT + MP, :], kT_d[:, 4:MT + MP, :])
        nc.sync.dma_start(va_t[:, :, 4:MT, :], va_d[:, :, 4:MT, :])
        for c in range(CT):
            nc.sync.dma_start(xT_t[c][:, QW:N],
                              xT_d[c * 128:(c + 1) * 128, QW:N])
        wp_t = big.tile([128, HPC, DIM], BF16)
        nc.sync.dma_start(wp_t[:], wp_d[:])

        qT_t = big.tile([128, 2, N], BF16)
        outTn_t = big.tile([128, HPC, N], BF16)
        zz_t = big.tile([128, 64], BF16)
        nc.vector.memset(zz_t[:], 0.0)

        acc_ps = ctx.enter_context(
            tc.tile_pool(name="acc_ps", bufs=2, space="PSUM"))
        st_stack = ExitStack()
        st_ps = st_stack.enter_context(
            tc.tile_pool(name="st_ps", bufs=2, space="PSUM"))

        qp_live = {}

        def qp_piece(s, ch, second):
            nsl = slice(ch * QW, (ch + 1) * QW)
            if not second:
                qp_live[(s, ch)] = acc_ps.tile(
                    [128, 2, QW], F32, tag="acc", name="acc", bufs=1)
            qp = qp_live[(s, ch)]
            for c in (range(3, CT) if second else range(3)):
                nc.tensor.matmul(
                    qp[:, 0, :], wq_t[:, s, c, :], xT_t[c][:, nsl],
                    start=(c == 0), stop=(c == CT - 1),
                )
            if second:
                nc.vector.tensor_copy(qT_t[:, s, nsl], qp[:, 0, :])
                del qp_live[(s, ch)]

        pj_live = {}

        def pj_piece(q, j, oc, second, pool, nbufs=1, dma_split=False):
            na = (q * 4 + 2 * j) * 128
            nb = na + 128
            osl = slice(oc * 384, (oc + 1) * 384)
            if not second:
                pj_live[(q, j, oc)] = pool.tile(
                    [128, 2, QW], F32, tag="acc", name="acc", bufs=nbufs)
            pjv = pj_live[(q, j, oc)][:, :, 0:384]
            for hh in ((2,) if second else (0, 1)):
                nc.tensor.matmul(
                    pjv[:, 0, :], outTn_t[0:64, hh, na:na + 128],
                    wp_t[0:64, hh, osl],
                    start=(hh == 0), stop=(hh == HPC - 1),
                    tile_position=(0, 0),
                )
                nc.tensor.matmul(
                    pjv[:, 1, :], outTn_t[64:128, hh, nb:nb + 128],
                    wp_t[64:128, hh, osl],
                    start=(hh == 0), stop=(hh == HPC - 1),
                    tile_position=(64, 0),
                )
            if second:
                ot = stg.tile([128, 2, 384], F16, tag="ot", name="ot")
                if (j + oc) % 2 == 0:
                    nc.scalar.copy(ot[:], pjv[:])
                else:
                    nc.vector.tensor_copy(ot[:], pjv[:])
                eng2 = nc.scalar if dma_split else nc.sync
                nc.sync.dma_start(out_d[na:na + 128, osl], ot[:, 0, :])
                eng2.dma_start(out_d[nb:nb + 128, osl], ot[:, 1, :])
                del pj_live[(q, j, oc)]

        # schedule: per 24-iter q-block, iters 0:16 are the h01-paired
        # unit and 16:24 the h2 unit.  The shared "acc" ring holds the h2
        # accumulator from block-iter 16 until the next block's iter ~1,
        # so injected pieces (qproj / proj) sit only in block slots 2..15.
        inject = {}

        def put(g, piece):
            assert g not in inject and 2 <= (g % 24) <= 15, g
            inject[g] = piece

        put(2, ("qp", 1, 0, False))
        put(3, ("qp", 1, 0, True))
        put(12, ("qp", 0, 1, False))
        put(13, ("qp", 0, 1, True))
        put(14, ("qp", 1, 1, False))
        put(15, ("qp", 1, 1, True))
        for q in range(NQ - 1):
            b = 24 * (q + 1) + 3
            for g in range(4):
                j, oc = g // 2, g % 2
                put(b + 2 * g, ("pj", q, j, oc, False))
                put(b + 2 * g + 1, ("pj", q, j, oc, True))
        put(35, ("qp", 0, 2, False))
        put(36, ("qp", 0, 2, True))
        put(37, ("qp", 1, 2, False))
        put(38, ("qp", 1, 2, True))
        put(59, ("qp", 0, 3, False))
        put(60, ("qp", 0, 3, True))
        put(61, ("qp", 1, 3, False))
        put(62, ("qp", 1, 3, True))

        av_by_key = {}
        pend = []

        def _av(pd):
            kind, q, idx, et, first, last = pd
            if kind == "P":
                nc.tensor.matmul(av_by_key[("P0", q)][:],
                                 va_t[:, 0, idx, :], et[:, 0, :],
                                 start=first, stop=last)
                nc.tensor.matmul(av_by_key[("P1", q)][:],
                                 va_t[:, 1, idx, :], et[:, 1, :],
                                 start=first, stop=last)
            else:
                av2 = av_by_key[("S", q)][:, 0, :]
                nc.tensor.matmul(av2, va_t[:, 2, 2 * idx, :], et[:, 0, :],
                                 start=first, stop=False)
                nc.tensor.matmul(av2, va_t[:, 2, 2 * idx + 1, :],
                                 et[:, 1, :], start=False, stop=last)

        def _norm_one(av, h, q):
            nsl = slice(q * QW, (q + 1) * QW)
            rs = rsp.tile([64, QW], F32, tag="rs", name="rs")
            nc.vector.reciprocal_approx_fast(rs[:], av[0:64, :])
            nc.vector.tensor_mul(outTn_t[0:64, h, nsl], av[64:128, :],
                                 rs[:])
            nc.sync.dma_start(outTn_t[64:128, h, nsl],
                              outTn_t[0:64, h, nsl])

        def _norm(kind, q):
            if kind == "P":
                _norm_one(av_by_key[("P0", q)][:], 0, q)
                _norm_one(av_by_key[("P1", q)][:], 1, q)
            else:
                _norm_one(av_by_key[("S", q)][:, 0, :], 2, q)

        def _flush(limit):
            while len(pend) > limit:
                pd = pend.pop(0)
                _av(pd)
                if pd[5]:
                    _norm(pd[0], pd[1])

        for w in range(2):
            wt = st_ps.tile([128, 2, QW], F32, tag="st", name="st")
            for i in range(20):
                nc.tensor.matmul(wt[0:64, 0, 0:64], zz_t[:, 0:64],
                                 zz_t[:, 0:64], start=True, stop=True)

        qp_piece(0, 0, False)
        qp_piece(0, 0, True)

        gi = 0
        for q in range(NQ):
            nsl = slice(q * QW, (q + 1) * QW)
            for kind, niter in (("P", MT), ("S", MP)):
                for it in range(niter):
                    if it == 0:
                        if kind == "P":
                            av_by_key[("P0", q)] = acc_ps.tile(
                                [128, QW], F32, tag="av", name="av")
                            av_by_key[("P1", q)] = acc_ps.tile(
                                [128, QW], F32, tag="av", name="av")
                        else:
                            av_by_key[("S", q)] = acc_ps.tile(
                                [128, 2, QW], F32, tag="acc", name="acc",
                                bufs=1)
                    kt = it if kind == "P" else MT + it
                    qs = 0 if kind == "P" else 1
                    st = st_ps.tile([128, 2, QW], F32, tag="st", name="st")
                    nc.tensor.matmul(
                        st[:, 0, :], kT_t[0:64, kt, :],
                        qT_t[0:64, qs, nsl],
                        start=True, stop=True, tile_position=(0, 0),
                    )
                    nc.tensor.matmul(
                        st[:, 1, :], kT_t[64:128, kt, :],
                        qT_t[64:128, qs, nsl],
                        start=True, stop=True, tile_position=(64, 0),
                    )
                    _flush(LAG - 1)
                    et = etp.tile([128, 2, QW], BF16, tag="et", name="et")
                    if (gi % 16) in _DVE_MOD and gi < 88:
                        nc.vector._custom_dve(
                            exp_op, out=et[:], in0=st[:],
                            s0=float(EA), s1=float(EB))
                    else:
                        nc.scalar.activation(
                            et[:], st[:], mybir.ActivationFunctionType.Exp,
                            scale=ACT_SCALE)
                    pend.append((kind, q, it, et, it == 0,
                                 it == niter - 1))
                    item = inject.get(gi)
                    if item is not None:
                        if item[0] == "qp":
                            qp_piece(item[1], item[2], item[3])
                        else:
                            pj_piece(item[1], item[2], item[3], item[4],
                                     acc_ps)
                    gi += 1
        _flush(0)
        st_stack.close()
        with tc.tile_pool(name="tail_ps", bufs=2, space="PSUM") as tailp:
            for jj in range(2):
                pj_piece(NQ - 1, jj, 0, False, tailp, 2)
                pj_piece(NQ - 1, jj, 1, False, tailp, 2)
                pj_piece(NQ - 1, jj, 0, True, tailp, 2, dma_split=True)
                pj_piece(NQ - 1, jj, 1, True, tailp, 2, dma_split=True)

    nc.compile()
    return nc


def build_in_maps(x, k, v, w_qkv, w_proj):
    x = np.asarray(x, dtype=np.float32)
    k = np.asarray(k, dtype=np.float32)
    v = np.asarray(v, dtype=np.float32)
    wqT = np.ascontiguousarray(np.asarray(w_qkv, np.float32).T) * SIGMA
    wpT = np.ascontiguousarray(np.asarray(w_proj, np.float32).T)
    bf = ml_dtypes.bfloat16

    in_maps = []
    for core in range(NCORES):
        b = core // 4
        hs = [3 * (core % 4) + i for i in range(HPC)]
        xT = np.ascontiguousarray(x[b].T.astype(bf))
        # wq slot 0: h0 cols 0:64, h1 cols 64:128; slot 1: h2 duplicated
        wq = np.empty((128, 2, CT, 128), dtype=bf)
        for sl, (ha, hb) in enumerate(((hs[0], hs[1]), (hs[2], hs[2]))):
            ba = wqT[:, 64 * ha:64 * (ha + 1)].reshape(CT, 128, 64)
            bb = wqT[:, 64 * hb:64 * (hb + 1)].reshape(CT, 128, 64)
            wq[:, sl, :, 0:64] = ba.transpose(1, 0, 2).astype(bf)
            wq[:, sl, :, 64:128] = bb.transpose(1, 0, 2).astype(bf)
        # kT: entries 0:16 pair h0 (rows 0:64) with h1 (rows 64:128) per
        # m-tile; entries 16:24 pack h2's even/odd m-tiles
        kT = np.empty((128, MT + MP, 128), dtype=bf)
        k0, k1, k2 = (k[b, hs[0]], k[b, hs[1]], k[b, hs[2]])
        for t in range(MT):
            kT[0:64, t, :] = k0[128 * t:128 * (t + 1), :].T
            kT[64:128, t, :] = k1[128 * t:128 * (t + 1), :].T
        for p in range(MP):
            kT[0:64, MT + p, :] = k2[256 * p:256 * p + 128, :].T
            kT[64:128, MT + p, :] = k2[256 * p + 128:256 * p + 256, :].T
        # va: cols 0:64 ones (denominator), 64:128 = v
        va = np.ones((128, HPC, MT, 128), dtype=bf)
        for hi, h in enumerate(hs):
            va[:, hi, :, D:2 * D] = v[b, h].reshape(MT, 128, D).transpose(
                1, 0, 2).astype(bf)
        wp = np.empty((128, HPC, DIM), dtype=bf)
        for hi, h in enumerate(hs):
            wp[0:64, hi, :] = wpT[64 * h:64 * (h + 1), :].astype(bf)
            wp[64:128, hi, :] = wp[0:64, hi, :]
        in_maps.append({"xT": xT, "wq": np.ascontiguousarray(wq),
                        "kT": np.ascontiguousarray(kT),
                        "va": np.ascontiguousarray(va),
                        "wp": np.ascontiguousarray(wp)})
    return in_maps


def kernel(x, k, v, w_qkv, w_proj, b_proj):
    b_proj = np.asarray(b_proj, dtype=np.float32)

    if "nc" not in _cached:
        _cached["nc"] = build_program()
    nc = _cached["nc"]

    in_maps = build_in_maps(x, k, v, w_qkv, w_proj)
    res = run_bass_kernel_spmd(nc, in_maps, core_ids=list(range(NCORES)))

    out = np.empty((B, N, DIM), dtype=np.float32)
    for b in range(B):
        acc = np.zeros((N, DIM), dtype=np.float64)
        for core in range(4 * b, 4 * b + 4):
            acc += res.results[core]["out"].astype(np.float64)
        out[b] = (acc + b_proj).astype(np.float32)
    return out
